# revision 10
# baseline (speedup 1.0000x reference)
"""DeepSet encoder (phi MLP -> sum/max pool -> rho MLP) as a Trainium2 Bass kernel.

Sharding: data-parallel over the batch dim. 64 samples -> 8 cores x 8 samples.
Weights are replicated on every core; no cross-core communication.

On-chip layout is feature-major ("transposed"): activations live as
[feature_partition, set_free] tiles so that
  - matmul contraction (over features) is on the partition dim,
  - the bias is a per-partition scalar (free on ScalarE's activation op),
  - sum/max pooling over the set dim is a free-axis reduction
    (sum comes for free via activation's accum_out).
The host pre-transposes x to [B, D_IN, N] and casts inputs to fp8/fp16.

v2 changes vs baseline (166.3us):
  - phi2 runs in fp8e4m3 DoubleRow too (h1 stored fp8, W2 staged fp8):
    halves phi2's PE row count (the dominant 112us of PE time).
  - bulk weights (w2/wr1/wr2/rho biases) stream on the ACT HWDGE ring,
    issued at t=0, so rho never stalls on its 6MB of weights and the SP
    ring only carries the latency-critical xt/w1/b loads.
  - startup DMAs target per-chunk tiles so the first LDWEIGHTS waits on
    256KB, not the whole xt0+w1 group.
  - epilogue work is balanced ScalarE/VectorE: ACT does the phi2
    relu+bias+sum-accum (it owns the accumulator), DVE does the max
    reduce (fp16 out, straight into pooled_bf) plus 5/8 of the phi1
    relu+bias->fp8 evacuations; ACT picks up the other 3/8.

Self-contained: only relies on the system-installed concourse/bass stack.
"""

import sys

import numpy as np

for _p in ("/opt/trn_rl_repo",):
    if _p not in sys.path:
        sys.path.insert(0, _p)

import ml_dtypes  # noqa: E402

import concourse.bass as bass  # noqa: E402,F401
import concourse.mybir as mybir  # noqa: E402
import concourse.tile as tile  # noqa: E402
from concourse import bacc  # noqa: E402
from concourse.bass_utils import run_bass_kernel_spmd  # noqa: E402

# fp16 carries 10 mantissa bits at the same PE rate as bf16; all intermediates
# here are O(100) max, far inside fp16 range.
BF16 = mybir.dt.float16
FP32 = mybir.dt.float32
NP_BF16 = np.float16
# phi1 AND phi2 run in fp8e4m3 with DoubleRow: 2 fp8 weights per PE cell
# contract 256 rows per pass. x ~ N(0,1), W1 ~ U(+-0.044), h1 in [0,~4],
# W2 ~ U(+-0.031) all sit far inside TRN e4m3's +-240 range. The fp8
# quantization noise in h2 is averaged down ~sqrt(512) by the sum-pool, and
# the max-pool half contributes ~1% of rho1's input magnitude, so end-to-end
# error stays ~0.005 against a 2e-2 gate.
FP8 = mybir.dt.float8e4
NP_FP8 = ml_dtypes.float8_e4m3
DOUBLE_ROW = mybir.MatmulPerfMode.DoubleRow

B, N, D_IN, D_H = 64, 512, 512, 1024
N_CORES = 8
BL = B // N_CORES  # samples per core
P = 128
K2 = D_H // P  # phi2/rho2 contraction tiles & D_H output tiles (8)
KR1 = 2 * D_H // P  # rho1 contraction tiles (16)
KK1 = D_IN // 256  # phi1 DoubleRow chunks (2)
KK2 = D_H // 256  # phi2 DoubleRow chunks (4)

RELU = mybir.ActivationFunctionType.Relu
AX_X = mybir.AxisListType.X
OP_MAX = mybir.AluOpType.max
OP_ADD = mybir.AluOpType.add


def build_program() -> bacc.Bacc:
    nc = bacc.Bacc("TRN2", target_bir_lowering=False, debug=False, num_devices=N_CORES)

    # all staged host-side into the exact SBUF tile layouts so every DMA is
    # contiguous per partition (large descriptor runs):
    #   xt[b, p, kk, j, n] = x[b, n, kk*256 + j*128 + p]   (fp8, DoubleRow pairs)
    #   w1[p, kk, j, h] = W1[kk*256 + j*128 + p, h]        (fp8)
    #   w2[p, kk, j, h] = W2[kk*256 + j*128 + p, h]        (fp8)
    #   w*[p, ko, h] = W[ko*128+p, h]                      (fp16)
    xt_d = nc.dram_tensor("xt", [BL, P, KK1, 2, N], FP8, kind="ExternalInput").ap()
    w1_d = nc.dram_tensor("w1", [P, KK1, 2, D_H], FP8, kind="ExternalInput").ap()
    w2_d = nc.dram_tensor("w2", [P, KK2, 2, D_H], FP8, kind="ExternalInput").ap()
    wr1_d = nc.dram_tensor("wr1", [P, KR1, D_H], BF16, kind="ExternalInput").ap()
    wr2_d = nc.dram_tensor("wr2", [P, K2, D_H], BF16, kind="ExternalInput").ap()
    # biases staged on host as [P, n_tiles]: b_sb[p, m] = b[m*128 + p]
    # phi1 epilogue re-centering: h1' = relu(ps+b1) - c = max(ps + (b1-c), -c),
    # with c a per-feature, fp8-representable estimate of E[h1]. Re-centering
    # kills the coherent (DC) path of W2's fp8 quantization error into the
    # sum-pool; the exact correction c @ W2_fp32 is folded into b2 on host.
    b1mc_d = nc.dram_tensor("b1mc", [P, K2], FP32, kind="ExternalInput").ap()
    negc_d = nc.dram_tensor("negc", [P, K2], FP32, kind="ExternalInput").ap()
    b2_d = nc.dram_tensor("b2", [P, K2], FP32, kind="ExternalInput").ap()
    br1_d = nc.dram_tensor("br1", [P, K2], FP32, kind="ExternalInput").ap()
    br2_d = nc.dram_tensor("br2", [P, K2], FP32, kind="ExternalInput").ap()
    # out[p, m, s] = r2[m*128 + p, s]  (feature-major, host transposes back)
    out_d = nc.dram_tensor("out", [P, K2, BL], FP32, kind="ExternalOutput").ap()

    with tile.TileContext(nc) as tc:
        with (
            tc.tile_pool(name="const", bufs=1) as cpool,
            tc.tile_pool(name="xt", bufs=3) as xtpool,
            tc.tile_pool(name="h1", bufs=2) as h1pool,
            tc.tile_pool(name="h2", bufs=4) as h2pool,
            tc.tile_pool(name="ps", bufs=8, space="PSUM") as pspool,
        ):
            # --- bulk weights on the ACT HWDGE ring, issued immediately ---
            # w2 (1MB) is needed by phi2(0) at ~15us; wr1/wr2 (6MB) stream in
            # the background during the phi loop so rho never waits on them.
            w2_sb = cpool.tile([P, KK2, 2, D_H], FP8)
            wr1_sb = cpool.tile([P, KR1, D_H], BF16)
            wr2_sb = cpool.tile([P, K2, D_H], BF16)
            br1_sb = cpool.tile([P, K2], FP32)
            br2_sb = cpool.tile([P, K2], FP32)

            # --- PE warm-up ---
            # The PE clock sits at 1.2GHz until ~5us of sustained activity.
            # Burn that window (which overlaps the DMA ring spin-up + first
            # transfers anyway) on dummy matmuls over a zeroed scratch tile,
            # so the real matmuls run at 2.4GHz from the first one.
            warm_sb = cpool.tile([P, N], BF16)
            nc.vector.memset(warm_sb[:], 0.0)
            for i in range(8):
                wps = pspool.tile([P, N], FP32, tag="ps", name=f"warm{i}")
                nc.tensor.matmul(wps[:], warm_sb[:, 0:P], warm_sb[:], start=True, stop=True)

            # --- latency-critical startup DMAs on the SP ring ---
            # Per-chunk tiles: the first LDWEIGHTS only waits for w1 chunk 0
            # (256KB) and the first matmul additionally for xt0 chunk 0.
            w1k_sb = [cpool.tile([P, 2, D_H], FP8, name=f"w1k{kk}") for kk in range(KK1)]
            xt0k_sb = [
                xtpool.tile([P, 2, N], FP8, tag="xt0", name=f"xt0k{kk}")
                for kk in range(KK1)
            ]
            nc.sync.dma_start(w1k_sb[0][:], w1_d[:, 0])
            nc.sync.dma_start(xt0k_sb[0][:], xt_d[0, :, 0])
            nc.sync.dma_start(w1k_sb[1][:], w1_d[:, 1])
            nc.sync.dma_start(xt0k_sb[1][:], xt_d[0, :, 1])
            b1mc_sb = cpool.tile([P, K2], FP32)
            nc.sync.dma_start(b1mc_sb[:], b1mc_d)
            negc_sb = cpool.tile([P, K2], FP32)
            nc.sync.dma_start(negc_sb[:], negc_d)
            b2_sb = cpool.tile([P, K2], FP32)
            nc.sync.dma_start(b2_sb[:], b2_d)
            xt1_sb = xtpool.tile([P, KK1, 2, N], FP8, tag="xt", name="xt1")
            nc.sync.dma_start(xt1_sb[:], xt_d[1])
            # w2 (1MB, needed ~8us later than w1/xt0) starts only after the
            # last critical startup chunk is down: WAW-gate via dummy write.
            nc.vector.tensor_copy(w2_sb[:, 0, 0, 0:1], xt0k_sb[1][:, 0, 0:1])
            nc.scalar.dma_start(w2_sb[:], w2_d)

            pooled = cpool.tile([P, K2, BL], FP32)  # sum-pool (fp32, ACT accum)
            pooled_bf = cpool.tile([P, KR1, BL], BF16)  # [0:K2]=sum, [K2:]=max
            r1_sb = cpool.tile([P, K2, BL], BF16)
            out_sb = cpool.tile([P, K2, BL], FP32)

            def phi1_mm(ps, m, kk, rhs, start, stop):
                # fp8 DoubleRow: lhsT [128, 2, 128], rhs [128, 2, 512];
                # contracts 256 input-feature rows per pass.
                nc.tensor.matmul(
                    ps[:],
                    w1k_sb[kk][:, :, m * P : (m + 1) * P],
                    rhs,
                    perf_mode=DOUBLE_ROW,
                    start=start,
                    stop=stop,
                )

            def h1_epilogue(h1_sb, m, ps):
                # max(ps + (b1-c), -c) = relu(ps+b1) - c -> fp8 h1 tile.
                # ACT can't express the -c floor, so chunks 3 and 6 skip
                # re-centering (c=0 staged host-side -> plain relu) and run
                # on ACT, relieving DVE which otherwise paces the PE.
                if m in (3, 6):
                    nc.scalar.activation(
                        h1_sb[:, m // 2, m % 2, :], ps[:], RELU,
                        bias=b1mc_sb[:, m : m + 1], scale=1.0,
                    )
                else:
                    nc.vector.tensor_scalar(
                        h1_sb[:, m // 2, m % 2, :], ps[:],
                        b1mc_sb[:, m : m + 1], negc_sb[:, m : m + 1],
                        OP_ADD, OP_MAX,
                    )

            def phi1(b):
                if b == 1:
                    xt_sb = xt1_sb
                elif b > 1:
                    xt_sb = xtpool.tile([P, KK1, 2, N], FP8, tag="xt", name=f"xt{b}")
                    nc.sync.dma_start(xt_sb[:], xt_d[b])
                h1_sb = h1pool.tile([P, KK2, 2, N], FP8, tag="h1", name=f"h1_{b}")
                if b == 0:
                    # two half-k accumulation waves across all m so the first
                    # 8 matmuls only need chunk 0 of the xt0/w1 DMAs.
                    ps1 = []
                    for m in range(K2):
                        ps = pspool.tile([P, N], FP32, tag="ps", name=f"ps1_0_{m}")
                        ps1.append(ps)
                        phi1_mm(ps, m, 0, xt0k_sb[0][:], start=True, stop=False)
                    for m in range(K2):
                        ps = ps1[m]
                        phi1_mm(ps, m, 1, xt0k_sb[1][:], start=False, stop=True)
                        h1_epilogue(h1_sb, m, ps)
                    return h1_sb
                for m in range(K2):
                    ps = pspool.tile([P, N], FP32, tag="ps", name=f"ps1_{b}_{m}")
                    for kk in range(KK1):
                        phi1_mm(
                            ps, m, kk, xt_sb[:, kk],
                            start=(kk == 0), stop=(kk == KK1 - 1),
                        )
                    h1_epilogue(h1_sb, m, ps)
                return h1_sb

            def phi2(b, h1_sb):
                h2_sb = None
                for m in range(K2):
                    ps = pspool.tile([P, N], FP32, tag="ps", name=f"ps2_{b}_{m}")
                    for kk in range(KK2):
                        # fp8 DoubleRow again: contracts 256 h1-feature rows
                        # per pass -> 4 matmuls per m-tile instead of 8.
                        nc.tensor.matmul(
                            ps[:],
                            w2_sb[:, kk, :, m * P : (m + 1) * P],
                            h1_sb[:, kk],
                            perf_mode=DOUBLE_ROW,
                            start=(kk == 0),
                            stop=(kk == KK2 - 1),
                        )
                    if m % 2 == 0:
                        h2_sb = h2pool.tile([P, 2, N], BF16, tag="h2", name=f"h2_{b}_{m}")
                    # relu(psum + bias) -> h2 half-tile; sum over set dim lands
                    # in pooled[:, m, b] via the activation accumulator.
                    nc.scalar.activation(
                        h2_sb[:, m % 2, :],
                        ps[:],
                        RELU,
                        bias=b2_sb[:, m : m + 1],
                        scale=1.0,
                        accum_out=pooled[:, m, b : b + 1],
                    )
                    if b == BL - 1:
                        # last sample: the sum feature tile is complete as soon
                        # as the ACT accumulator lands -> cast it on ACT itself
                        # (no queueing behind DVE reduces) so rho1's sum-half
                        # matmuls can start immediately.
                        nc.scalar.activation(
                            pooled_bf[:, m, :], pooled[:, m, :],
                            mybir.ActivationFunctionType.Copy,
                        )
                    if b == BL - 1:
                        # last sample: emit singles so each max lands as soon
                        # as its h2 half-tile does; rho1's max-half chases.
                        nc.vector.tensor_reduce(
                            pooled_bf[:, K2 + m, b : b + 1],
                            h2_sb[:, m % 2, :],
                            axis=AX_X,
                            op=OP_MAX,
                        )
                    elif m % 2 == 1:
                        # paired max-pool straight to fp16 (max is exact; fp16
                        # just rounds): one DVE op covers two feature tiles.
                        nc.vector.tensor_reduce(
                            pooled_bf[:, K2 + m - 1 : K2 + m + 1, b : b + 1],
                            h2_sb[:],
                            axis=AX_X,
                            op=OP_MAX,
                        )

            # software pipeline: phi1(b+1) is emitted before phi2(b) so the PE
            # never waits on the phi1->phi2 evacuation inside one sample.
            prev_h1 = None
            for b in range(BL):
                h1_sb = phi1(b)
                if prev_h1 is not None:
                    phi2(b - 1, prev_h1)
                    if b == 1:
                        # Start streaming the 6MB of rho weights only once
                        # phi2(0) is underway. The tile scheduler hoists
                        # dep-free DMAs to t=0 (where they starve the
                        # latency-critical startup loads and trip the HAM
                        # activity throttle), so give each bulk DMA a WAW
                        # dependency via a 1-element dummy write that hangs
                        # off phi2(0)'s first pooled accumulator.
                        nc.vector.tensor_copy(wr1_sb[:, 0, 0:1], pooled[:, 0, 0:1])
                        nc.scalar.dma_start(wr1_sb[:], wr1_d)
                        nc.vector.tensor_copy(wr2_sb[:, 0, 0:1], pooled[:, 0, 0:1])
                        nc.scalar.dma_start(wr2_sb[:], wr2_d)
                        nc.vector.tensor_copy(br1_sb[:, 0:1], pooled[:, 0, 0:1])
                        nc.scalar.dma_start(br1_sb[:], br1_d)
                        nc.vector.tensor_copy(br2_sb[:, 0:1], pooled[:, 0, 0:1])
                        nc.scalar.dma_start(br2_sb[:], br2_d)
                prev_h1 = h1_sb
            phi2(BL - 1, prev_h1)

            # --- rho MLP over the 8 pooled vectors (feature-major, N=8) ---
            # rho1 in two half-accumulations over all 8 m-tiles: the sum-half
            # (k=0..7) only needs the ACT accumulators, so its matmuls chase
            # the phi2 epilogue while the max reduces are still draining.
            ps_r1 = []
            for m in range(K2):
                ps = pspool.tile([P, BL], FP32, tag="ps", name=f"psr1_{m}")
                ps_r1.append(ps)
                for k in range(K2):
                    nc.tensor.matmul(
                        ps[:],
                        wr1_sb[:, k, m * P : (m + 1) * P],
                        pooled_bf[:, k, :],
                        start=(k == 0),
                        stop=False,
                    )
            for m in range(K2):
                ps = ps_r1[m]
                for k in range(K2, KR1):
                    nc.tensor.matmul(
                        ps[:],
                        wr1_sb[:, k, m * P : (m + 1) * P],
                        pooled_bf[:, k, :],
                        start=False,
                        stop=(k == KR1 - 1),
                    )
                # alternate evacuations between ScalarE and VectorE so the
                # short rho epilogue isn't serialized on one engine; DVE does
                # max(x + bias, 0) in a single tensor_scalar op.
                if m % 2 == 0:
                    nc.scalar.activation(
                        r1_sb[:, m, :], ps[:], RELU,
                        bias=br1_sb[:, m : m + 1], scale=1.0,
                    )
                else:
                    nc.vector.tensor_scalar(
                        r1_sb[:, m, :], ps[:],
                        br1_sb[:, m : m + 1], 0.0,
                        OP_ADD, OP_MAX,
                    )
            for m in range(K2):
                ps = pspool.tile([P, BL], FP32, tag="ps", name=f"psr2_{m}")
                for k in range(K2):
                    nc.tensor.matmul(
                        ps[:],
                        wr2_sb[:, k, m * P : (m + 1) * P],
                        r1_sb[:, k, :],
                        start=(k == 0),
                        stop=(k == K2 - 1),
                    )
                if m % 2 == 0:
                    nc.scalar.activation(
                        out_sb[:, m, :], ps[:], RELU,
                        bias=br2_sb[:, m : m + 1], scale=1.0,
                    )
                else:
                    nc.vector.tensor_scalar(
                        out_sb[:, m, :], ps[:],
                        br2_sb[:, m : m + 1], 0.0,
                        OP_ADD, OP_MAX,
                    )
                if m == K2 // 2 - 1:
                    # first half of the output leaves while rho2 finishes
                    nc.sync.dma_start(out_d[:, : K2 // 2], out_sb[:, : K2 // 2])
            # second half goes out on the ACT ring so the two DIRECT2D
            # issues (~0.6us each) overlap instead of serializing.
            nc.scalar.dma_start(out_d[:, K2 // 2 :], out_sb[:, K2 // 2 :])

    return nc


_CACHE: dict = {}


def get_compiled() -> bacc.Bacc:
    if "nc" not in _CACHE:
        nc = build_program()
        nc.compile()
        _CACHE["nc"] = nc
    return _CACHE["nc"]


def stage_inputs(x, W_phi1, b_phi1, W_phi2, b_phi2, W_rho1, b_rho1, W_rho2, b_rho2):
    """Host-side staging: transpose x, cast to fp8/fp16, reshape biases."""

    def wtile(a):
        # [KO*P, H] -> [P, KO, H] with w[p, ko, h] = W[ko*P + p, h]
        a = np.asarray(a, np.float32).astype(NP_BF16)
        ko = a.shape[0] // P
        return np.ascontiguousarray(a.reshape(ko, P, -1).transpose(1, 0, 2))

    def wtile8(a):
        # [KK*256, H] -> [P, KK, 2, H] with w[p, kk, j, h] = W[kk*256+j*128+p, h]
        a = np.asarray(a, np.float32).astype(NP_FP8)
        kk = a.shape[0] // 256
        return np.ascontiguousarray(a.reshape(kk, 2, P, -1).transpose(2, 0, 1, 3))

    def bias(a):
        # [n_tiles*P] -> [P, n_tiles] with b_sb[p, m] = b[m*P + p]
        return np.ascontiguousarray(np.asarray(a, np.float32).reshape(-1, P).T)

    # x[b, n, d] -> xt[b, p, kk, j, n] = x[b, n, kk*256 + j*128 + p]  (fp8)
    xt = np.asarray(x, np.float32).astype(NP_FP8)
    xt = np.ascontiguousarray(xt.reshape(B, N, KK1, 2, P).transpose(0, 4, 2, 3, 1))
    # Re-centering constant c[k] ~ E_n[h1[n, k]] for x ~ N(0, I):
    # h1_pre[., k] ~ N(b1[k], ||W1[:, k]||^2), so E[relu] has a closed form.
    # Snapped to the fp8 grid so the exact-zero h1 entries (about half, from
    # the relu) quantize to -c with NO rounding error.
    from math import erf as _erf

    W1f = np.asarray(W_phi1, np.float32)
    b1f = np.asarray(b_phi1, np.float32)
    sig = np.linalg.norm(W1f, axis=0)
    a = b1f / sig
    cdf = 0.5 * (1.0 + np.vectorize(_erf)(a / np.sqrt(2.0)))
    pdf = np.exp(-0.5 * a * a) / np.sqrt(2.0 * np.pi)
    c = (sig * pdf + b1f * cdf).astype(np.float32)
    c = c.astype(NP_FP8).astype(np.float32)  # fp8-representable
    # chunks 3 and 6 run their epilogue on ScalarE (plain relu, no -c floor)
    c[3 * P : 4 * P] = 0.0
    c[6 * P : 7 * P] = 0.0
    b2_corr = np.asarray(b_phi2, np.float32) + c @ np.asarray(W_phi2, np.float32)
    shared = {
        "w1": wtile8(W_phi1),
        "w2": wtile8(W_phi2),
        "wr1": wtile(W_rho1),
        "wr2": wtile(W_rho2),
        "b1mc": bias(b1f - c),
        "negc": bias(-c),
        "b2": bias(b2_corr),
        "br1": bias(b_rho1),
        "br2": bias(b_rho2),
    }
    in_maps = []
    for c in range(N_CORES):
        m = dict(shared)
        m["xt"] = np.ascontiguousarray(xt[c * BL : (c + 1) * BL])
        in_maps.append(m)
    return in_maps


def gather_output(results) -> np.ndarray:
    # per-core out: [P, K2, BL] with out[p, m, s] = r2[m*128+p, s]
    parts = []
    for c in range(N_CORES):
        o = np.asarray(results[c]["out"], np.float32)  # [P, K2, BL]
        parts.append(o.transpose(2, 1, 0).reshape(BL, D_H))  # [BL, D_H]
    return np.concatenate(parts, axis=0)


def run(trace: bool = False, **inputs):
    nc = get_compiled()
    in_maps = stage_inputs(**inputs)
    res = run_bass_kernel_spmd(nc, in_maps, core_ids=list(range(N_CORES)), trace=trace)
    return gather_output(res.results), res


def kernel(**inputs) -> np.ndarray:
    out, _ = run(trace=False, **inputs)
    return out


# revision 11
# speedup vs baseline: 1.0277x; 1.0277x over previous
"""DeepSet encoder (phi MLP -> sum/max pool -> rho MLP) as a Trainium2 Bass kernel.

Sharding: data-parallel over the batch dim. 64 samples -> 8 cores x 8 samples.
Weights are replicated on every core; no cross-core communication.

On-chip layout is feature-major ("transposed"): activations live as
[feature_partition, set_free] tiles so that
  - matmul contraction (over features) is on the partition dim,
  - the bias is a per-partition scalar (free on ScalarE's activation op),
  - sum/max pooling over the set dim is a free-axis reduction
    (sum comes for free via activation's accum_out).
The host pre-transposes x to [B, D_IN, N] and casts inputs to fp8/fp16.

v2 changes vs baseline (166.3us):
  - phi2 runs in fp8e4m3 DoubleRow too (h1 stored fp8, W2 staged fp8):
    halves phi2's PE row count (the dominant 112us of PE time).
  - bulk weights (w2/wr1/wr2/rho biases) stream on the ACT HWDGE ring,
    issued at t=0, so rho never stalls on its 6MB of weights and the SP
    ring only carries the latency-critical xt/w1/b loads.
  - startup DMAs target per-chunk tiles so the first LDWEIGHTS waits on
    256KB, not the whole xt0+w1 group.
  - epilogue work is balanced ScalarE/VectorE: ACT does the phi2
    relu+bias+sum-accum (it owns the accumulator), DVE does the max
    reduce (fp16 out, straight into pooled_bf) plus 5/8 of the phi1
    relu+bias->fp8 evacuations; ACT picks up the other 3/8.

Self-contained: only relies on the system-installed concourse/bass stack.
"""

import sys

import numpy as np

for _p in ("/opt/trn_rl_repo",):
    if _p not in sys.path:
        sys.path.insert(0, _p)

import ml_dtypes  # noqa: E402

import concourse.bass as bass  # noqa: E402,F401
import concourse.mybir as mybir  # noqa: E402
import concourse.tile as tile  # noqa: E402
from concourse import bacc  # noqa: E402
from concourse.bass_utils import run_bass_kernel_spmd  # noqa: E402

# fp16 carries 10 mantissa bits at the same PE rate as bf16; all intermediates
# here are O(100) max, far inside fp16 range.
BF16 = mybir.dt.float16
FP32 = mybir.dt.float32
NP_BF16 = np.float16
# phi1 AND phi2 run in fp8e4m3 with DoubleRow: 2 fp8 weights per PE cell
# contract 256 rows per pass. x ~ N(0,1), W1 ~ U(+-0.044), h1 in [0,~4],
# W2 ~ U(+-0.031) all sit far inside TRN e4m3's +-240 range. The fp8
# quantization noise in h2 is averaged down ~sqrt(512) by the sum-pool, and
# the max-pool half contributes ~1% of rho1's input magnitude, so end-to-end
# error stays ~0.005 against a 2e-2 gate.
FP8 = mybir.dt.float8e4
NP_FP8 = ml_dtypes.float8_e4m3
DOUBLE_ROW = mybir.MatmulPerfMode.DoubleRow

B, N, D_IN, D_H = 64, 512, 512, 1024
N_CORES = 8
BL = B // N_CORES  # samples per core
P = 128
K2 = D_H // P  # phi2/rho2 contraction tiles & D_H output tiles (8)
KR1 = 2 * D_H // P  # rho1 contraction tiles (16)
KK1 = D_IN // 256  # phi1 DoubleRow chunks (2)
KK2 = D_H // 256  # phi2 DoubleRow chunks (4)

RELU = mybir.ActivationFunctionType.Relu
AX_X = mybir.AxisListType.X
OP_MAX = mybir.AluOpType.max
OP_ADD = mybir.AluOpType.add


def build_program() -> bacc.Bacc:
    nc = bacc.Bacc("TRN2", target_bir_lowering=False, debug=False, num_devices=N_CORES)

    # all staged host-side into the exact SBUF tile layouts so every DMA is
    # contiguous per partition (large descriptor runs):
    #   xt[b, p, kk, j, n] = x[b, n, kk*256 + j*128 + p]   (fp8, DoubleRow pairs)
    #   w1[p, kk, j, h] = W1[kk*256 + j*128 + p, h]        (fp8)
    #   w2[p, kk, j, h] = W2[kk*256 + j*128 + p, h]        (fp8)
    #   w*[p, ko, h] = W[ko*128+p, h]                      (fp16)
    xt_d = nc.dram_tensor("xt", [BL, P, KK1, 2, N], FP8, kind="ExternalInput").ap()
    w1_d = nc.dram_tensor("w1", [P, KK1, 2, D_H], FP8, kind="ExternalInput").ap()
    w2_d = nc.dram_tensor("w2", [P, KK2, 2, D_H], FP8, kind="ExternalInput").ap()
    wr1_d = nc.dram_tensor("wr1", [P, KR1, D_H], BF16, kind="ExternalInput").ap()
    wr2_d = nc.dram_tensor("wr2", [P, K2, D_H], BF16, kind="ExternalInput").ap()
    # biases staged on host as [P, n_tiles]: b_sb[p, m] = b[m*128 + p]
    # phi1 epilogue re-centering: h1' = relu(ps+b1) - c = max(ps + (b1-c), -c),
    # with c a per-feature, fp8-representable estimate of E[h1]. Re-centering
    # kills the coherent (DC) path of W2's fp8 quantization error into the
    # sum-pool; the exact correction c @ W2_fp32 is folded into b2 on host.
    b1mc_d = nc.dram_tensor("b1mc", [P, K2], FP32, kind="ExternalInput").ap()
    negc_d = nc.dram_tensor("negc", [P, K2], FP32, kind="ExternalInput").ap()
    b2_d = nc.dram_tensor("b2", [P, K2], FP32, kind="ExternalInput").ap()
    br1_d = nc.dram_tensor("br1", [P, K2], FP32, kind="ExternalInput").ap()
    br2_d = nc.dram_tensor("br2", [P, K2], FP32, kind="ExternalInput").ap()
    # out[p, m, s] = r2[m*128 + p, s]  (feature-major, host transposes back)
    out_d = nc.dram_tensor("out", [P, K2, BL], FP32, kind="ExternalOutput").ap()

    with tile.TileContext(nc) as tc:
        with (
            tc.tile_pool(name="const", bufs=1) as cpool,
            tc.tile_pool(name="xt", bufs=3) as xtpool,
            tc.tile_pool(name="h1", bufs=2) as h1pool,
            tc.tile_pool(name="h2", bufs=4) as h2pool,
            tc.tile_pool(name="ps", bufs=8, space="PSUM") as pspool,
        ):
            # --- bulk weights on the ACT HWDGE ring, issued immediately ---
            # w2 (1MB) is needed by phi2(0) at ~15us; wr1/wr2 (6MB) stream in
            # the background during the phi loop so rho never waits on them.
            w2_sb = cpool.tile([P, KK2, 2, D_H], FP8)
            wr1_sb = cpool.tile([P, KR1, D_H], BF16)
            wr2_sb = cpool.tile([P, K2, D_H], BF16)
            br1_sb = cpool.tile([P, K2], FP32)
            br2_sb = cpool.tile([P, K2], FP32)

            # --- PE warm-up ---
            # The PE clock sits at 1.2GHz until ~5us of sustained activity.
            # Burn that window (which overlaps the DMA ring spin-up + first
            # transfers anyway) on dummy matmuls over a zeroed scratch tile,
            # so the real matmuls run at 2.4GHz from the first one.
            warm_sb = cpool.tile([P, N], BF16)
            nc.vector.memset(warm_sb[:], 0.0)
            for i in range(8):
                wps = pspool.tile([P, N], FP32, tag="ps", name=f"warm{i}")
                nc.tensor.matmul(wps[:], warm_sb[:, 0:P], warm_sb[:], start=True, stop=True)

            # --- latency-critical startup DMAs on the SP ring ---
            # Per-chunk tiles: the first LDWEIGHTS only waits for w1 chunk 0
            # (256KB) and the first matmul additionally for xt0 chunk 0.
            w1k_sb = [cpool.tile([P, 2, D_H], FP8, name=f"w1k{kk}") for kk in range(KK1)]
            xt0k_sb = [
                xtpool.tile([P, 2, N], FP8, tag="xt0", name=f"xt0k{kk}")
                for kk in range(KK1)
            ]
            nc.sync.dma_start(w1k_sb[0][:], w1_d[:, 0])
            nc.sync.dma_start(xt0k_sb[0][:], xt_d[0, :, 0])
            nc.sync.dma_start(w1k_sb[1][:], w1_d[:, 1])
            nc.sync.dma_start(xt0k_sb[1][:], xt_d[0, :, 1])
            xt1_sb = xtpool.tile([P, KK1, 2, N], FP8, tag="xt", name="xt1")
            nc.sync.dma_start(xt1_sb[:], xt_d[1])
            b1mc_sb = cpool.tile([P, K2], FP32)
            nc.sync.dma_start(b1mc_sb[:], b1mc_d)
            negc_sb = cpool.tile([P, K2], FP32)
            nc.sync.dma_start(negc_sb[:], negc_d)
            b2_sb = cpool.tile([P, K2], FP32)
            nc.sync.dma_start(b2_sb[:], b2_d)
            xt2_sb = xtpool.tile([P, KK1, 2, N], FP8, tag="xt", name="xt2")
            nc.sync.dma_start(xt2_sb[:], xt_d[2])
            # w2 (1MB, needed ~8us later than w1/xt0) starts only after the
            # last critical startup chunk is down: WAW-gate via dummy write.
            nc.vector.tensor_copy(w2_sb[:, 0, 0, 0:1], xt0k_sb[1][:, 0, 0:1])
            nc.scalar.dma_start(w2_sb[:], w2_d)

            pooled = cpool.tile([P, K2, BL], FP32)  # sum-pool (fp32, ACT accum)
            pooled_bf = cpool.tile([P, KR1, BL], BF16)  # [0:K2]=sum, [K2:]=max
            r1_sb = cpool.tile([P, K2, BL], BF16)
            out_sb = cpool.tile([P, K2, BL], FP32)

            def phi1_mm(ps, m, kk, rhs, start, stop):
                # fp8 DoubleRow: lhsT [128, 2, 128], rhs [128, 2, 512];
                # contracts 256 input-feature rows per pass.
                nc.tensor.matmul(
                    ps[:],
                    w1k_sb[kk][:, :, m * P : (m + 1) * P],
                    rhs,
                    perf_mode=DOUBLE_ROW,
                    start=start,
                    stop=stop,
                )

            def h1_epilogue(h1_sb, m, ps):
                # max(ps + (b1-c), -c) = relu(ps+b1) - c -> fp8 h1 tile.
                # ACT can't express the -c floor, so these all live on DVE
                # (gpsimd can't: TensorScalarPtr has no Pool-engine ucode).
                nc.vector.tensor_scalar(
                    h1_sb[:, m // 2, m % 2, :], ps[:],
                    b1mc_sb[:, m : m + 1], negc_sb[:, m : m + 1],
                    OP_ADD, OP_MAX,
                )

            def phi1(b):
                if b == 1:
                    xt_sb = xt1_sb
                elif b == 2:
                    xt_sb = xt2_sb
                elif b > 2:
                    xt_sb = xtpool.tile([P, KK1, 2, N], FP8, tag="xt", name=f"xt{b}")
                    nc.sync.dma_start(xt_sb[:], xt_d[b])
                h1_sb = h1pool.tile([P, KK2, 2, N], FP8, tag="h1", name=f"h1_{b}")
                if b == 0:
                    # two half-k accumulation waves across all m so the first
                    # 8 matmuls only need chunk 0 of the xt0/w1 DMAs.
                    ps1 = []
                    for m in range(K2):
                        ps = pspool.tile([P, N], FP32, tag="ps", name=f"ps1_0_{m}")
                        ps1.append(ps)
                        phi1_mm(ps, m, 0, xt0k_sb[0][:], start=True, stop=False)
                    for m in range(K2):
                        ps = ps1[m]
                        phi1_mm(ps, m, 1, xt0k_sb[1][:], start=False, stop=True)
                        h1_epilogue(h1_sb, m, ps)
                    return h1_sb
                for m in range(K2):
                    ps = pspool.tile([P, N], FP32, tag="ps", name=f"ps1_{b}_{m}")
                    for kk in range(KK1):
                        phi1_mm(
                            ps, m, kk, xt_sb[:, kk],
                            start=(kk == 0), stop=(kk == KK1 - 1),
                        )
                    h1_epilogue(h1_sb, m, ps)
                return h1_sb

            def phi2(b, h1_sb):
                h2_sb = None
                for m in range(K2):
                    ps = pspool.tile([P, N], FP32, tag="ps", name=f"ps2_{b}_{m}")
                    for kk in range(KK2):
                        # fp8 DoubleRow again: contracts 256 h1-feature rows
                        # per pass -> 4 matmuls per m-tile instead of 8.
                        nc.tensor.matmul(
                            ps[:],
                            w2_sb[:, kk, :, m * P : (m + 1) * P],
                            h1_sb[:, kk],
                            perf_mode=DOUBLE_ROW,
                            start=(kk == 0),
                            stop=(kk == KK2 - 1),
                        )
                    if m % 2 == 0:
                        h2_sb = h2pool.tile([P, 2, N], BF16, tag="h2", name=f"h2_{b}_{m}")
                    # relu(psum + bias) -> h2 half-tile; sum over set dim lands
                    # in pooled[:, m, b] via the activation accumulator.
                    nc.scalar.activation(
                        h2_sb[:, m % 2, :],
                        ps[:],
                        RELU,
                        bias=b2_sb[:, m : m + 1],
                        scale=1.0,
                        accum_out=pooled[:, m, b : b + 1],
                    )
                    if b == BL - 1:
                        # last sample: cast the completed sum tile on DVE,
                        # emitted ahead of the max reduce so rho1's sum-half
                        # matmuls chase it; ACT stays free for the epilogues
                        # (it is the critical path of the phi2(7) drain).
                        nc.vector.tensor_copy(pooled_bf[:, m, :], pooled[:, m, :])
                    if m % 2 == 1:
                        # paired max-pool straight to fp16 (max is exact; fp16
                        # just rounds): one DVE op covers two feature tiles.
                        nc.vector.tensor_reduce(
                            pooled_bf[:, K2 + m - 1 : K2 + m + 1, b : b + 1],
                            h2_sb[:],
                            axis=AX_X,
                            op=OP_MAX,
                        )

            # software pipeline: phi1(b+1) is emitted before phi2(b) so the PE
            # never waits on the phi1->phi2 evacuation inside one sample.
            prev_h1 = None
            for b in range(BL):
                h1_sb = phi1(b)
                if prev_h1 is not None:
                    phi2(b - 1, prev_h1)
                    if b == 1:
                        # Start streaming the 6MB of rho weights only once
                        # phi2(0) is underway. The tile scheduler hoists
                        # dep-free DMAs to t=0 (where they starve the
                        # latency-critical startup loads and trip the HAM
                        # activity throttle), so give each bulk DMA a WAW
                        # dependency via a 1-element dummy write that hangs
                        # off phi2(0)'s first pooled accumulator.
                        nc.vector.tensor_copy(wr1_sb[:, 0, 0:1], pooled[:, 0, 0:1])
                        nc.scalar.dma_start(wr1_sb[:], wr1_d)
                        nc.vector.tensor_copy(wr2_sb[:, 0, 0:1], pooled[:, 0, 0:1])
                        nc.scalar.dma_start(wr2_sb[:], wr2_d)
                        nc.vector.tensor_copy(br1_sb[:, 0:1], pooled[:, 0, 0:1])
                        nc.scalar.dma_start(br1_sb[:], br1_d)
                        nc.vector.tensor_copy(br2_sb[:, 0:1], pooled[:, 0, 0:1])
                        nc.scalar.dma_start(br2_sb[:], br2_d)
                prev_h1 = h1_sb
            phi2(BL - 1, prev_h1)

            # --- rho MLP over the 8 pooled vectors (feature-major, N=8) ---
            # rho1 in two half-accumulations over all 8 m-tiles: the sum-half
            # (k=0..7) only needs the ACT accumulators, so its matmuls chase
            # the phi2 epilogue while the max reduces are still draining.
            ps_r1 = []
            for m in range(K2):
                ps = pspool.tile([P, BL], FP32, tag="ps", name=f"psr1_{m}")
                ps_r1.append(ps)
                for k in range(K2):
                    nc.tensor.matmul(
                        ps[:],
                        wr1_sb[:, k, m * P : (m + 1) * P],
                        pooled_bf[:, k, :],
                        start=(k == 0),
                        stop=False,
                    )
            for m in range(K2):
                ps = ps_r1[m]
                for k in range(K2, KR1):
                    nc.tensor.matmul(
                        ps[:],
                        wr1_sb[:, k, m * P : (m + 1) * P],
                        pooled_bf[:, k, :],
                        start=False,
                        stop=(k == KR1 - 1),
                    )
                # alternate evacuations between ScalarE and VectorE so the
                # short rho epilogue isn't serialized on one engine; DVE does
                # max(x + bias, 0) in a single tensor_scalar op.
                if m % 2 == 0:
                    nc.scalar.activation(
                        r1_sb[:, m, :], ps[:], RELU,
                        bias=br1_sb[:, m : m + 1], scale=1.0,
                    )
                else:
                    nc.vector.tensor_scalar(
                        r1_sb[:, m, :], ps[:],
                        br1_sb[:, m : m + 1], 0.0,
                        OP_ADD, OP_MAX,
                    )
            for m in range(K2):
                ps = pspool.tile([P, BL], FP32, tag="ps", name=f"psr2_{m}")
                for k in range(K2):
                    nc.tensor.matmul(
                        ps[:],
                        wr2_sb[:, k, m * P : (m + 1) * P],
                        r1_sb[:, k, :],
                        start=(k == 0),
                        stop=(k == K2 - 1),
                    )
                if m % 2 == 0:
                    nc.scalar.activation(
                        out_sb[:, m, :], ps[:], RELU,
                        bias=br2_sb[:, m : m + 1], scale=1.0,
                    )
                else:
                    nc.vector.tensor_scalar(
                        out_sb[:, m, :], ps[:],
                        br2_sb[:, m : m + 1], 0.0,
                        OP_ADD, OP_MAX,
                    )
                if m == K2 // 2 - 1:
                    # first half of the output leaves while rho2 finishes
                    nc.sync.dma_start(out_d[:, : K2 // 2], out_sb[:, : K2 // 2])
            # second half goes out on the ACT ring so the two DIRECT2D
            # issues (~0.6us each) overlap instead of serializing.
            nc.scalar.dma_start(out_d[:, K2 // 2 :], out_sb[:, K2 // 2 :])

    return nc


_CACHE: dict = {}


def get_compiled() -> bacc.Bacc:
    if "nc" not in _CACHE:
        nc = build_program()
        nc.compile()
        _CACHE["nc"] = nc
    return _CACHE["nc"]


def stage_inputs(x, W_phi1, b_phi1, W_phi2, b_phi2, W_rho1, b_rho1, W_rho2, b_rho2):
    """Host-side staging: transpose x, cast to fp8/fp16, reshape biases."""

    def wtile(a):
        # [KO*P, H] -> [P, KO, H] with w[p, ko, h] = W[ko*P + p, h]
        a = np.asarray(a, np.float32).astype(NP_BF16)
        ko = a.shape[0] // P
        return np.ascontiguousarray(a.reshape(ko, P, -1).transpose(1, 0, 2))

    def wtile8(a):
        # [KK*256, H] -> [P, KK, 2, H] with w[p, kk, j, h] = W[kk*256+j*128+p, h]
        a = np.asarray(a, np.float32).astype(NP_FP8)
        kk = a.shape[0] // 256
        return np.ascontiguousarray(a.reshape(kk, 2, P, -1).transpose(2, 0, 1, 3))

    def bias(a):
        # [n_tiles*P] -> [P, n_tiles] with b_sb[p, m] = b[m*P + p]
        return np.ascontiguousarray(np.asarray(a, np.float32).reshape(-1, P).T)

    # x[b, n, d] -> xt[b, p, kk, j, n] = x[b, n, kk*256 + j*128 + p]  (fp8)
    xt = np.asarray(x, np.float32).astype(NP_FP8)
    xt = np.ascontiguousarray(xt.reshape(B, N, KK1, 2, P).transpose(0, 4, 2, 3, 1))
    # Re-centering constant c[k] ~ E_n[h1[n, k]] for x ~ N(0, I):
    # h1_pre[., k] ~ N(b1[k], ||W1[:, k]||^2), so E[relu] has a closed form.
    # Snapped to the fp8 grid so the exact-zero h1 entries (about half, from
    # the relu) quantize to -c with NO rounding error.
    from math import erf as _erf

    W1f = np.asarray(W_phi1, np.float32)
    b1f = np.asarray(b_phi1, np.float32)
    sig = np.linalg.norm(W1f, axis=0)
    a = b1f / sig
    cdf = 0.5 * (1.0 + np.vectorize(_erf)(a / np.sqrt(2.0)))
    pdf = np.exp(-0.5 * a * a) / np.sqrt(2.0 * np.pi)
    c = (sig * pdf + b1f * cdf).astype(np.float32)
    c = c.astype(NP_FP8).astype(np.float32)  # fp8-representable
    b2_corr = np.asarray(b_phi2, np.float32) + c @ np.asarray(W_phi2, np.float32)
    shared = {
        "w1": wtile8(W_phi1),
        "w2": wtile8(W_phi2),
        "wr1": wtile(W_rho1),
        "wr2": wtile(W_rho2),
        "b1mc": bias(b1f - c),
        "negc": bias(-c),
        "b2": bias(b2_corr),
        "br1": bias(b_rho1),
        "br2": bias(b_rho2),
    }
    in_maps = []
    for c in range(N_CORES):
        m = dict(shared)
        m["xt"] = np.ascontiguousarray(xt[c * BL : (c + 1) * BL])
        in_maps.append(m)
    return in_maps


def gather_output(results) -> np.ndarray:
    # per-core out: [P, K2, BL] with out[p, m, s] = r2[m*128+p, s]
    parts = []
    for c in range(N_CORES):
        o = np.asarray(results[c]["out"], np.float32)  # [P, K2, BL]
        parts.append(o.transpose(2, 1, 0).reshape(BL, D_H))  # [BL, D_H]
    return np.concatenate(parts, axis=0)


def run(trace: bool = False, **inputs):
    nc = get_compiled()
    in_maps = stage_inputs(**inputs)
    res = run_bass_kernel_spmd(nc, in_maps, core_ids=list(range(N_CORES)), trace=trace)
    return gather_output(res.results), res


def kernel(**inputs) -> np.ndarray:
    out, _ = run(trace=False, **inputs)
    return out
